# revision 20
# baseline (speedup 1.0000x reference)
"""Multi-head attention (B=4, S=2048, D=1024, H=16) on 8 trn2 NeuronCores.

Sharding: (batch, head-half) -> one core each. Core c handles batch c//2 and
heads (c%2)*8 .. (c%2)*8+7 (feature columns (c%2)*512 .. +512 of the QKV
projections, rows of Wo). Each core computes its 8 heads' attention and a
partial output projection; the host sums the two partials per batch and adds
the output bias.

v4 schedule (vs the 540us baseline):
  - steady state is paced by the ACT engine (256 exps x ~1.05us); scores are
    issued 3 units ahead across group boundaries so ACT rides through PE
    bursts, and all chased projection work is split into 4-matmul halves
    spread over adjacent slots,
  - input DMA: quarter-granularity [128, 512] transfers in exact consumption
    order, split over the two usable DGE queues (sync HW + gpsimd SW; the
    scalar engine carries none, its FIFO would block ACTIVATEs),
  - softmax denominators of both heads are packed at partitions 0/64 of one
    [65, 512] tile -> a single DVE reciprocal per group,
  - wo partials merge into [128, 1024] stores alternating between queues.
Matmuls run in bf16 (fp32 PSUM accumulation); softmax denominators and
reciprocals stay fp32.
"""
import numpy as np

import concourse.bass as bass
import concourse.tile as tile
from concourse import mybir
from concourse.bass_utils import run_bass_kernel_spmd

F32 = mybir.dt.float32
F32R = mybir.dt.float32r
BF16 = mybir.dt.bfloat16
EXP = mybir.ActivationFunctionType.Exp

B, S, DM, H_TOT = 4, 2048, 1024, 16
F = 512          # features per core (8 heads x 64)
HD = 64          # head dim
NH = 8           # heads per core
NP = 4           # head pairs per core
KT = 16          # k tiles of 128
NQT = 4          # q chunks of 512
SCALE = 0.125    # 1/sqrt(64)
N_CORES = 8
LOOK = 3         # scores issued LOOK units ahead

_WAIT_CAP = {"InstEventSemaphore": 2}


def _split_multiwaits(nc):
    """This walrus build accepts 1 sync-wait per instruction (2 on
    EventSemaphore); spread extras over same-engine NOPs placed before."""
    n_spill = 0
    for f in nc.m.functions:
        for bb in f.blocks:
            new = []
            changed = False
            for inst in bb.instructions:
                si = inst.sync_info
                cap = _WAIT_CAP.get(type(inst).__name__, 1)
                if si is not None and len(si.on_wait) > cap:
                    extra = list(si.on_wait[: len(si.on_wait) - cap])
                    del si.on_wait[: len(si.on_wait) - cap]
                    for w in extra:
                        n_spill += 1
                        nop = mybir.InstNoOp(name=f"I-wspill-{n_spill}-{inst.name}")
                        nop.engine = inst.engine
                        nop.sync_info = mybir.SyncInfo(on_wait=[w], on_update=[])
                        new.append(nop)
                    changed = True
                new.append(inst)
            if changed:
                bb.instructions[:] = new
    return n_spill


def build_program():
    nc = bass.Bass("TRN2", target_bir_lowering=False, debug=False, num_devices=1)

    # super-layout inputs: row p of each [128, 4096] tensor holds the
    # per-128-block rows concatenated, so one DMA moves 1 MB with 8 KB
    # contiguous lines (341 GB/s vs ~90 GB/s for 0.125 MB transfers)
    d_qt = [nc.dram_tensor(f"qt{n}", [128, 4096], BF16, kind="ExternalInput").ap() for n in range(4)]
    d_kt = [nc.dram_tensor(f"kt{n}", [128, 4096], BF16, kind="ExternalInput").ap() for n in range(4)]
    d_vt = [nc.dram_tensor(f"vt{n}", [128, 4096], BF16, kind="ExternalInput").ap() for n in range(4)]
    d_wq = nc.dram_tensor("wq", [128, 4096], BF16, kind="ExternalInput").ap()
    d_wk = nc.dram_tensor("wk", [128, 4096], BF16, kind="ExternalInput").ap()
    d_wv = nc.dram_tensor("wv", [128, 4096], BF16, kind="ExternalInput").ap()
    d_wo = nc.dram_tensor("wo", [128, 4096], BF16, kind="ExternalInput").ap()
    d_bq = nc.dram_tensor("bq", [F], F32, kind="ExternalInput").ap()
    d_bk = nc.dram_tensor("bk", [F], F32, kind="ExternalInput").ap()
    d_bv = nc.dram_tensor("bv", [F], F32R, kind="ExternalInput").ap()
    d_ones = nc.dram_tensor("ones", [65, 128], F32R, kind="ExternalInput").ap()
    d_part = nc.dram_tensor("part", [S, DM], F32, kind="ExternalOutput").ap()

    with tile.TileContext(nc) as tc:
        with (
            tc.tile_pool(name="wpool", bufs=1) as wpool,
            tc.tile_pool(name="big", bufs=1) as big,
            tc.tile_pool(name="oTp", bufs=2) as oTp,
            tc.tile_pool(name="ktst", bufs=1) as ktst,
            tc.tile_pool(name="qtst", bufs=1) as qtst,
            tc.tile_pool(name="vtst", bufs=2) as vtst,
            tc.tile_pool(name="exch", bufs=4) as exch,
            tc.tile_pool(name="outst", bufs=2) as outst,
            tc.tile_pool(name="rcp", bufs=2) as rcp,
            tc.tile_pool(name="ocp", bufs=4) as ocp,
            tc.tile_pool(name="ps_sc", bufs=2, space="PSUM") as ps_sc,
            tc.tile_pool(name="ps_pv", bufs=2, space="PSUM") as ps_pv,
            tc.tile_pool(name="ps_acc", bufs=2, space="PSUM") as ps_acc,
        ):
            # ---- resident weight tiles (one merged DMA each)
            wq_sb = wpool.tile([128, 8 * F], BF16, tag="wq")
            wk_sb = wpool.tile([128, 8 * F], BF16, tag="wk")
            wv_sb = wpool.tile([128, 8 * F], BF16, tag="wv")
            wo_sb = wpool.tile([128, 4 * DM], BF16, tag="wo")
            bq_sb = wpool.tile([128, 4], F32, tag="bq")
            bk_sb = wpool.tile([128, 4], F32, tag="bk")
            bv_sb = wpool.tile([1, F], F32R, tag="bv")
            ones_sb = wpool.tile([65, 128], F32R, tag="ones")
            bvbc_sb = wpool.tile([128, F], F32, tag="bvbc")
            warm_sb = wpool.tile([1, 2], F32, tag="warm")
            dn2_sb = wpool.tile([65, 512], F32, tag="dn2")

            # big attention tiles: qT/kT hold the full token range per pair;
            # oT rotates per q-chunk (wo consumption finishes within 1 chunk)
            qT_sb = [big.tile([128, S], BF16, tag=f"qT{f}", name=f"qT{f}") for f in range(4)]
            kT_sb = [big.tile([128, S], BF16, tag=f"kT{f}", name=f"kT{f}") for f in range(4)]
            v_sb = [big.tile([128, NH * (HD + 1)], BF16, tag=f"v{t}", name=f"v{t}") for t in range(KT)]

            # ---- DMA: one 1 MB instruction per staged chunk.  The
            # scores-critical tier-1 set rides the gpsimd (SW DGE) queue
            # which empirically wins the SDMA round-robin; the sync queue's
            # transfers are gate-held (1-elem gpsimd copies create the
            # cross-queue ordering) so arrivals match consumption order.
            nc.sync.dma_start(ones_sb[:], d_ones[:])
            nc.sync.dma_start(bq_sb[:], d_bq.rearrange("(f p) -> p f", p=128))
            nc.sync.dma_start(bk_sb[:], d_bk.rearrange("(f p) -> p f", p=128))
            nc.sync.dma_start(bv_sb[:], d_bv.rearrange("(a f) -> a f", a=1))
            kt_t, qt_t, vt_t = {}, {}, {}

            def gate(dst_t, src_t):
                nc.gpsimd.tensor_copy(dst_t[0:1, 0:1], src_t[0:1, 0:1])

            # tier 1 on gpsimd: everything the first scores/exp need
            nc.gpsimd.dma_start(wk_sb[:], d_wk[:])
            kt_t[0] = ktst.tile([128, 4096], BF16, tag="kt0", name="kts0")
            nc.gpsimd.dma_start(kt_t[0][:], d_kt[0][:])
            nc.gpsimd.dma_start(wq_sb[:], d_wq[:])
            qt_t[0] = qtst.tile([128, 4096], BF16, tag="qt0", name="qts0")
            nc.gpsimd.dma_start(qt_t[0][:], d_qt[0][:])
            # wv on sync, gated on qt0 (tier 2)
            gate(wv_sb, qt_t[0])
            nc.sync.dma_start(wv_sb[:], d_wv[:])
            # vt chunks stream on gpsimd (implicitly after tier 1);
            # kt1..3 on sync, each gated one vt behind
            for n in range(4):
                tv = vtst.tile([128, 4096], BF16, tag=f"vt{n % 2}", name=f"vts{n}")
                nc.gpsimd.dma_start(tv[:], d_vt[n][:])
                vt_t[n] = tv
                if n < 3:
                    kt_t[n + 1] = ktst.tile([128, 4096], BF16, tag=f"kt{n + 1}", name=f"kts{n + 1}")
                    gate(kt_t[n + 1], tv)
                    nc.sync.dma_start(kt_t[n + 1][:], d_kt[n + 1][:])
            gate(wo_sb, vt_t[3])
            nc.sync.dma_start(wo_sb[:], d_wo[:])
            qt_t[1] = qtst.tile([128, 4096], BF16, tag="qt1", name="qts1")
            nc.gpsimd.dma_start(qt_t[1][:], d_qt[1][:])

            def load_late_qt():
                # late q chunks reuse the vt staging buffers; emitted only
                # after every v piece (the buffers' readers) exists
                for n in range(2, 4):
                    t_ = vtst.tile([128, 4096], BF16, tag=f"vt{n % 2}", name=f"qts{n}")
                    nc.sync.dma_start(t_[:], d_qt[n][:])
                    qt_t[n] = t_

            # preload the exp+ln table set while DMA streams
            nc.scalar.activation(warm_sb[:], ones_sb[0:1, 0:2], EXP)
            nc.scalar.activation(warm_sb[:], ones_sb[0:1, 0:2],
                                 mybir.ActivationFunctionType.Ln)

            # denominators of both heads live at partitions 0 and 64 of one
            # [65, 512] tile so a single reciprocal covers both (cost is
            # free-size bound); rows 1..63 are don't-care
            nc.vector.memset(dn2_sb[:], 1.0)

            # bv broadcast over partitions via K=1 matmul
            psbv = ps_acc.tile([128, 512], F32, tag="pacc", name="psbv")
            nc.tensor.matmul(psbv[:], ones_sb[0:1, :], bv_sb[0:1, :])
            nc.vector.tensor_copy(bvbc_sb[:], psbv[:])

            # ---- projection pieces: emitted as two 4-matmul halves --------
            def kq_halves(src_t, w_sb, bias_sb, dst_sb, n, f):
                st = {}

                def h1():
                    accp = ps_acc.tile([128, 512], F32, tag="pacc", name=f"acc{n}{f}")
                    st["acc"] = accp
                    for m in range(4):
                        nc.tensor.matmul(
                            accp[:],
                            w_sb[:, 512 * m + 128 * f:512 * m + 128 * (f + 1)],
                            src_t[n][:, 512 * m:512 * (m + 1)],
                            start=(m == 0), stop=False,
                        )

                def h2():
                    accp = st["acc"]
                    for m in range(4, 8):
                        nc.tensor.matmul(
                            accp[:],
                            w_sb[:, 512 * m + 128 * f:512 * m + 128 * (f + 1)],
                            src_t[n][:, 512 * m:512 * (m + 1)],
                            start=False, stop=(m == 7),
                        )
                    with nc.allow_low_precision(reason="bf16 qT/kT store"):
                        nc.vector.tensor_scalar_add(
                            dst_sb[f][:, 512 * n:512 * (n + 1)],
                            accp[:],
                            bias_sb[:, f:f + 1],
                        )
                return h1, h2

            def kq_piece(src_t, w_sb, bias_sb, dst_sb, n, f):
                h1, h2 = kq_halves(src_t, w_sb, bias_sb, dst_sb, n, f)
                h1()
                h2()

            def v_piece(t):
                q4, o = t // 4, 128 * (t % 4)
                acc = ps_acc.tile([128, 512], F32, tag="pacc", name=f"accv{t}")
                for m in range(8):
                    nc.tensor.matmul(
                        acc[:], vt_t[q4][:, 512 * m + o:512 * m + o + 128],
                        wv_sb[:, 512 * m:512 * (m + 1)],
                        start=(m == 0), stop=(m == 7),
                    )
                v3 = v_sb[t][:].rearrange("p (h e) -> p h e", e=HD + 1)
                nc.vector.memset(v3[:, :, HD:HD + 1], 1.0)
                nc.vector.tensor_add(
                    v3[:, :, 0:HD],
                    acc[:].rearrange("p (h e) -> p h e", e=HD),
                    bvbc_sb[:].rearrange("p (h e) -> p h e", e=HD),
                )

            # ---- wo output projection pieces (chased) ------------------
            wo_pending = []
            oT_cur = [None] * 4
            wo_ost = {}
            wo_nq = [0]

            def emit_wo(count):
                # one call = one [128, 512] half; the merged [128, 1024] store
                # fires after the second half, alternating sync/gpsimd queues
                for _ in range(count):
                    if not wo_pending:
                        return
                    oTs, tt, j = wo_pending.pop(0)
                    pw = ps_acc.tile([128, 512], F32, tag="pacc", name="pw")
                    for f in range(4):
                        nc.tensor.matmul(
                            pw[:], oTs[f][:, 128 * (tt % 4):128 * (tt % 4 + 1)],
                            wo_sb[:, 1024 * f + 512 * j:1024 * f + 512 * (j + 1)],
                            start=(f == 0), stop=(f == 3),
                        )
                    if tt not in wo_ost:
                        wo_ost[tt] = outst.tile([128, 1024], F32, tag="outst", name=f"ost{tt}")
                    ost = wo_ost[tt]
                    nc.vector.tensor_copy(ost[:, 512 * j:512 * (j + 1)], pw[:])
                    if j == 1:
                        eng = nc.sync if wo_nq[0] % 2 == 0 else nc.gpsimd
                        wo_nq[0] += 1
                        eng.dma_start(
                            d_part[128 * tt:128 * (tt + 1), :], wo_ost.pop(tt)[:]
                        )

            # ---- per-group normalization, pipelined ---------------------
            def norm_p1(po, nm):
                oc = ocp.tile([65, 512], F32, tag="oc", name=nm)
                nc.vector.tensor_copy(oc[:], po[0:65, :])
                return oc

            def norm_recip(ocA, ocB):
                # 1/d = exp(-ln d) on the ACT engine (same table set as the
                # softmax exp), freeing the DVE of 3.3us iterative divides
                nc.vector.tensor_copy(dn2_sb[0:1, :], ocA[64:65, :])
                nc.vector.tensor_copy(dn2_sb[64:65, :], ocB[64:65, :])
                ln2 = rcp.tile([65, 512], F32, tag="ln", name="ln2")
                rc2 = rcp.tile([65, 512], F32R, tag="rc", name="rc2")
                with nc.allow_low_precision(reason="recip via exp(-ln d)"):
                    nc.scalar.activation(ln2[:], dn2_sb[:],
                                         mybir.ActivationFunctionType.Ln)
                    nc.scalar.activation(rc2[:], ln2[:], EXP, scale=-1.0)
                return rc2

            def norm_bcast(rc2, i):
                # broadcast head i's reciprocal row to a base-0 [64, 512]
                pb = ps_acc.tile([128, 512], F32, tag="pacc", name=f"pb{i}")
                r = 64 * i
                nc.tensor.matmul(pb[0:64, :], ones_sb[r:r + 1, 0:64], rc2[r:r + 1, :])
                return pb

            def norm_mul(oc, pb, oTf, i):
                with nc.allow_low_precision(reason="bf16 normalized out"):
                    nc.vector.tensor_mul(
                        oTf[64 * i:64 * i + 64, :], oc[0:64, :], pb[0:64, :],
                    )

            # ---- minimal serial head: what group (0,0) m=0..3 needs
            for n4 in range(NQT):
                kq_piece(kt_t, wk_sb, bk_sb, kT_sb, n4, 0)
            kq_piece(qt_t, wq_sb, bq_sb, qT_sb, 0, 0)
            for t in range(4):
                v_piece(t)

            # ---- flat attention pipeline over all (group, m) units -----
            groups = [(n, p) for n in range(NQT) for p in range(NP)]

            def sc_emit(gi, m):
                n, p = groups[gi]
                qsl = slice(512 * n, 512 * (n + 1))
                scp = ps_sc.tile([128, 1024], F32, tag="sc")
                ksl = slice(128 * m, 128 * (m + 1))
                nc.tensor.matmul(
                    scp[:, 0:512], kT_sb[p][0:64, ksl], qT_sb[p][0:64, qsl],
                    tile_position=(0, 0),
                )
                nc.tensor.matmul(
                    scp[:, 512:1024], kT_sb[p][64:128, ksl],
                    qT_sb[p][64:128, qsl], tile_position=(64, 0),
                )
                ex = exch.tile([128, 1024], BF16, tag="ex")
                nc.scalar.activation(ex[:], scp[:], EXP, scale=SCALE)
                return ex

            # chase-slot tables.  kq pieces appear as (h1 slot, h2 slot);
            # deadlines account for the LOOK=3 score lookahead.
            def chase_slots(n, p):
                sl = {}

                def add(m, fn):
                    sl.setdefault(m, []).append(fn)

                if n == 0:
                    if p == 0:
                        h1, h2 = kq_halves(kt_t, wk_sb, bk_sb, kT_sb, 0, 1)
                        add(1, h1); add(2, h2)
                        for t in range(4, 16):
                            add(t - 1, lambda t=t: v_piece(t))
                        h1, h2 = kq_halves(kt_t, wk_sb, bk_sb, kT_sb, 1, 1)
                        add(5, h1); add(6, h2)
                        h1, h2 = kq_halves(qt_t, wq_sb, bq_sb, qT_sb, 0, 1)
                        add(10, h1); add(11, h2)
                    else:
                        h1, h2 = kq_halves(kt_t, wk_sb, bk_sb, kT_sb, 2, p)
                        add(0, h1); add(1, h2)
                        h1, h2 = kq_halves(kt_t, wk_sb, bk_sb, kT_sb, 3, p)
                        add(2, h1); add(3, h2)
                        if p < 3:
                            h1, h2 = kq_halves(kt_t, wk_sb, bk_sb, kT_sb, 0, p + 1)
                            add(4, h1); add(5, h2)
                            h1, h2 = kq_halves(kt_t, wk_sb, bk_sb, kT_sb, 1, p + 1)
                            add(6, h1); add(7, h2)
                            h1, h2 = kq_halves(qt_t, wq_sb, bq_sb, qT_sb, 0, p + 1)
                            add(10, h1); add(11, h2)
                if n + 1 < NQT:
                    h1, h2 = kq_halves(qt_t, wq_sb, bq_sb, qT_sb, n + 1, p)
                    add(10, h1) if (n == 0 and p == 0) else add(8, h1)
                    add(12, h2) if (n == 0 and p == 0) else add(9, h2)
                return sl

            NU = len(groups) * KT
            exq = {}
            for u0 in range(LOOK):
                exq[u0] = sc_emit(u0 // KT, u0 % KT)
            prev_state = None
            norm_st = {}

            for gi, (n, p) in enumerate(groups):
                if gi == 1:
                    load_late_qt()
                if p == 0:
                    oT_cur = [oTp.tile([128, 512], BF16, tag=f"oT{f}", name=f"oT{f}c{n}")
                              for f in range(4)]
                oTs = oT_cur
                poA = ps_pv.tile([128, 512], F32, tag="po", name=f"poA{gi}")
                poB = ps_pv.tile([128, 512], F32, tag="po", name=f"poB{gi}")
                slots = chase_slots(n, p)
                for m in range(KT):
                    u = gi * KT + m
                    if u + LOOK < NU:
                        exq[u + LOOK] = sc_emit((u + LOOK) // KT, (u + LOOK) % KT)
                    ex = exq.pop(u)
                    nc.tensor.matmul(
                        poA[0:65, :], v_sb[m][:, 130 * p:130 * p + 65],
                        ex[:, 0:512], start=(m == 0), stop=(m == KT - 1),
                    )
                    nc.tensor.matmul(
                        poB[0:65, :], v_sb[m][:, 130 * p + 65:130 * p + 130],
                        ex[:, 512:1024], start=(m == 0), stop=(m == KT - 1),
                    )
                    # previous group's norm pipeline in this group's slack
                    if prev_state is not None:
                        pT, pp = prev_state
                        if m == 2:
                            norm_st["rc2"] = norm_recip(norm_st["ocA"], norm_st["ocB"])
                        elif m == 5:
                            norm_st["pbA"] = norm_bcast(norm_st["rc2"], 0)
                        elif m == 6:
                            norm_mul(norm_st["ocA"], norm_st["pbA"], pT[pp], 0)
                        elif m == 7:
                            norm_st["pbB"] = norm_bcast(norm_st["rc2"], 1)
                        elif m == 8:
                            norm_mul(norm_st["ocB"], norm_st["pbB"], pT[pp], 1)
                    for fn in slots.get(m, ()):
                        fn()
                    if m in (9, 10, 12, 13, 14):
                        emit_wo(1)
                # evacuate this group's PV psum right after the stop matmuls
                norm_st["ocA"] = norm_p1(poA, f"ocA{gi}")
                norm_st["ocB"] = norm_p1(poB, f"ocB{gi}")
                prev_state = (oTs, p)
                if p == NP - 1:
                    for t in range(4):
                        for j in range(2):
                            wo_pending.append((oTs, 4 * n + t, j))

            # ---- tail: last group's norms + remaining wo
            pT, pp = prev_state
            rc2 = norm_recip(norm_st["ocA"], norm_st["ocB"])
            pbA = norm_bcast(rc2, 0)
            norm_mul(norm_st["ocA"], pbA, pT[pp], 0)
            pbB = norm_bcast(rc2, 1)
            norm_mul(norm_st["ocB"], pbB, pT[pp], 1)
            emit_wo(len(wo_pending))

    _split_multiwaits(nc)
    return nc


_PROGRAM = None


def _get_program():
    global _PROGRAM
    if _PROGRAM is None:
        _PROGRAM = build_program()
    return _PROGRAM


def _sup(x2d):
    # [nb*128, C] -> [128, nb*C]: row p holds the 128-row blocks' p-th rows
    # concatenated, so a [128, C]-block view is x[:, C*m:C*(m+1)] and one
    # DMA moves the whole tensor with C-sized contiguous lines.
    nb, C = x2d.shape[0] // 128, x2d.shape[1]
    return np.ascontiguousarray(
        x2d.reshape(nb, 128, C).swapaxes(0, 1).reshape(128, nb * C))


def make_in_maps(Q, K, V, Wq, bq, Wk, bk, Wv, bv, Wo, bo):
    import ml_dtypes
    bf = lambda x: np.asarray(x, dtype=np.float32).astype(ml_dtypes.bfloat16)
    f32 = lambda x: np.asarray(x, dtype=np.float32)
    Q, K, V = bf(Q), bf(K), bf(V)
    Wq, Wk, Wv, Wo = bf(Wq), bf(Wk), bf(Wv), bf(Wo)
    bq, bk, bv = f32(bq), f32(bk), f32(bv)
    ones = np.ones((65, 128), np.float32)
    in_maps = []
    for c in range(N_CORES):
        b, hh = c // 2, c % 2
        fs = slice(F * hh, F * (hh + 1))
        qt, kt, vt = Q[b].T, K[b].T, V[b].T
        im = {
            "wq": _sup(Wq[:, fs]),
            "wk": _sup(Wk[:, fs]),
            "wv": _sup(Wv[:, fs]),
            "wo": _sup(Wo[fs, :]),
            "bq": np.ascontiguousarray(bq[fs]),
            "bk": np.ascontiguousarray(bk[fs]),
            "bv": np.ascontiguousarray(bv[fs]),
            "ones": ones,
        }
        for n in range(4):
            sl = slice(512 * n, 512 * (n + 1))
            im[f"qt{n}"] = _sup(qt[:, sl])
            im[f"kt{n}"] = _sup(kt[:, sl])
            im[f"vt{n}"] = _sup(vt[:, sl])
        in_maps.append(im)
    return in_maps


def kernel(Q, K, V, Wq, bq, Wk, bk, Wv, bv, Wo, bo, _trace=False, _trace_kwargs=None):
    nc = _get_program()
    in_maps = make_in_maps(Q, K, V, Wq, bq, Wk, bk, Wv, bv, Wo, bo)
    res = run_bass_kernel_spmd(
        nc, in_maps, core_ids=list(range(N_CORES)),
        trace=_trace, **(_trace_kwargs or {}),
    )
    parts = [r["part"] for r in res.results]
    out = np.stack([parts[2 * b] + parts[2 * b + 1] for b in range(B)])
    out += np.asarray(bo, dtype=np.float32)[None, None, :]
    if _trace:
        return out, res
    return out


# revision 21
# speedup vs baseline: 1.0010x; 1.0010x over previous
"""Multi-head attention (B=4, S=2048, D=1024, H=16) on 8 trn2 NeuronCores.

Sharding: (batch, head-half) -> one core each. Core c handles batch c//2 and
heads (c%2)*8 .. (c%2)*8+7 (feature columns (c%2)*512 .. +512 of the QKV
projections, rows of Wo). Each core computes its 8 heads' attention and a
partial output projection; the host sums the two partials per batch and adds
the output bias.

v4 schedule (vs the 540us baseline):
  - steady state is paced by the ACT engine (256 exps x ~1.05us); scores are
    issued 3 units ahead across group boundaries so ACT rides through PE
    bursts, and all chased projection work is split into 4-matmul halves
    spread over adjacent slots,
  - input DMA: quarter-granularity [128, 512] transfers in exact consumption
    order, split over the two usable DGE queues (sync HW + gpsimd SW; the
    scalar engine carries none, its FIFO would block ACTIVATEs),
  - softmax denominators of both heads are packed at partitions 0/64 of one
    [65, 512] tile -> a single DVE reciprocal per group,
  - wo partials merge into [128, 1024] stores alternating between queues.
Matmuls run in bf16 (fp32 PSUM accumulation); softmax denominators and
reciprocals stay fp32.
"""
import numpy as np

import concourse.bass as bass
import concourse.tile as tile
from concourse import mybir
from concourse.bass_utils import run_bass_kernel_spmd

F32 = mybir.dt.float32
F32R = mybir.dt.float32r
BF16 = mybir.dt.bfloat16
EXP = mybir.ActivationFunctionType.Exp

B, S, DM, H_TOT = 4, 2048, 1024, 16
F = 512          # features per core (8 heads x 64)
HD = 64          # head dim
NH = 8           # heads per core
NP = 4           # head pairs per core
KT = 16          # k tiles of 128
NQT = 4          # q chunks of 512
SCALE = 0.125    # 1/sqrt(64)
N_CORES = 8
LOOK = 3         # scores issued LOOK units ahead

_WAIT_CAP = {"InstEventSemaphore": 2}


def _split_multiwaits(nc):
    """This walrus build accepts 1 sync-wait per instruction (2 on
    EventSemaphore); spread extras over same-engine NOPs placed before."""
    n_spill = 0
    for f in nc.m.functions:
        for bb in f.blocks:
            new = []
            changed = False
            for inst in bb.instructions:
                si = inst.sync_info
                cap = _WAIT_CAP.get(type(inst).__name__, 1)
                if si is not None and len(si.on_wait) > cap:
                    extra = list(si.on_wait[: len(si.on_wait) - cap])
                    del si.on_wait[: len(si.on_wait) - cap]
                    for w in extra:
                        n_spill += 1
                        nop = mybir.InstNoOp(name=f"I-wspill-{n_spill}-{inst.name}")
                        nop.engine = inst.engine
                        nop.sync_info = mybir.SyncInfo(on_wait=[w], on_update=[])
                        new.append(nop)
                    changed = True
                new.append(inst)
            if changed:
                bb.instructions[:] = new
    return n_spill


def build_program():
    nc = bass.Bass("TRN2", target_bir_lowering=False, debug=False, num_devices=1)

    # super-layout inputs: row p of each [128, 4096] tensor holds the
    # per-128-block rows concatenated, so one DMA moves 1 MB with 8 KB
    # contiguous lines (341 GB/s vs ~90 GB/s for 0.125 MB transfers)
    d_qt = [nc.dram_tensor(f"qt{n}", [128, 4096], BF16, kind="ExternalInput").ap() for n in range(4)]
    d_kt = [nc.dram_tensor(f"kt{n}", [128, 4096], BF16, kind="ExternalInput").ap() for n in range(4)]
    d_vt = [nc.dram_tensor(f"vt{n}", [128, 4096], BF16, kind="ExternalInput").ap() for n in range(4)]
    d_wq = nc.dram_tensor("wq", [128, 4096], BF16, kind="ExternalInput").ap()
    d_wk = nc.dram_tensor("wk", [128, 4096], BF16, kind="ExternalInput").ap()
    d_wv = nc.dram_tensor("wv", [128, 4096], BF16, kind="ExternalInput").ap()
    d_wo = nc.dram_tensor("wo", [128, 4096], BF16, kind="ExternalInput").ap()
    d_bq = nc.dram_tensor("bq", [F], F32, kind="ExternalInput").ap()
    d_bk = nc.dram_tensor("bk", [F], F32, kind="ExternalInput").ap()
    d_bv = nc.dram_tensor("bv", [F], F32R, kind="ExternalInput").ap()
    d_ones = nc.dram_tensor("ones", [65, 128], F32R, kind="ExternalInput").ap()
    d_part = nc.dram_tensor("part", [S, DM], F32, kind="ExternalOutput").ap()

    with tile.TileContext(nc) as tc:
        with (
            tc.tile_pool(name="wpool", bufs=1) as wpool,
            tc.tile_pool(name="big", bufs=1) as big,
            tc.tile_pool(name="oTp", bufs=2) as oTp,
            tc.tile_pool(name="ktst", bufs=1) as ktst,
            tc.tile_pool(name="qtst", bufs=1) as qtst,
            tc.tile_pool(name="vtst", bufs=2) as vtst,
            tc.tile_pool(name="exch", bufs=4) as exch,
            tc.tile_pool(name="outst", bufs=2) as outst,
            tc.tile_pool(name="rcp", bufs=2) as rcp,
            tc.tile_pool(name="ocp", bufs=4) as ocp,
            tc.tile_pool(name="ps_sc", bufs=2, space="PSUM") as ps_sc,
            tc.tile_pool(name="ps_pv", bufs=2, space="PSUM") as ps_pv,
            tc.tile_pool(name="ps_acc", bufs=2, space="PSUM") as ps_acc,
        ):
            # ---- resident weight tiles (one merged DMA each)
            wq_sb = wpool.tile([128, 8 * F], BF16, tag="wq")
            wk_sb = wpool.tile([128, 8 * F], BF16, tag="wk")
            wv_sb = wpool.tile([128, 8 * F], BF16, tag="wv")
            wo_sb = wpool.tile([128, 4 * DM], BF16, tag="wo")
            bq_sb = wpool.tile([128, 4], F32, tag="bq")
            bk_sb = wpool.tile([128, 4], F32, tag="bk")
            bv_sb = wpool.tile([1, F], F32R, tag="bv")
            ones_sb = wpool.tile([65, 128], F32R, tag="ones")
            bvbc_sb = wpool.tile([128, F], F32, tag="bvbc")
            warm_sb = wpool.tile([1, 2], F32, tag="warm")
            dn2_sb = wpool.tile([65, 512], F32, tag="dn2")

            # big attention tiles: qT/kT hold the full token range per pair;
            # oT rotates per q-chunk (wo consumption finishes within 1 chunk)
            qT_sb = [big.tile([128, S], BF16, tag=f"qT{f}", name=f"qT{f}") for f in range(4)]
            kT_sb = [big.tile([128, S], BF16, tag=f"kT{f}", name=f"kT{f}") for f in range(4)]
            v_sb = [big.tile([128, NH * (HD + 1)], BF16, tag=f"v{t}", name=f"v{t}") for t in range(KT)]

            # ---- DMA: one 1 MB instruction per staged chunk, in two
            # concurrency tiers (many in-flight transfers are needed to
            # reach aggregate HBM rate; a single transfer only sustains
            # ~90 GB/s).  Tier 1 = everything group (0,0)'s start needs,
            # split across both queues; one gate per queue holds tier 2.
            nc.sync.dma_start(ones_sb[:], d_ones[:])
            nc.sync.dma_start(bq_sb[:], d_bq.rearrange("(f p) -> p f", p=128))
            nc.sync.dma_start(bk_sb[:], d_bk.rearrange("(f p) -> p f", p=128))
            nc.sync.dma_start(bv_sb[:], d_bv.rearrange("(a f) -> a f", a=1))
            kt_t, qt_t, vt_t = {}, {}, {}

            def gate(dst_t, src_t):
                nc.gpsimd.tensor_copy(dst_t[0:1, 0:1], src_t[0:1, 0:1])

            # tier 1 (concurrent): wk,kt0 on sync | wq,qt0,wv,vt0 on gpsimd
            nc.sync.dma_start(wk_sb[:], d_wk[:])
            kt_t[0] = ktst.tile([128, 4096], BF16, tag="kt0", name="kts0")
            nc.sync.dma_start(kt_t[0][:], d_kt[0][:])
            nc.gpsimd.dma_start(wq_sb[:], d_wq[:])
            qt_t[0] = qtst.tile([128, 4096], BF16, tag="qt0", name="qts0")
            nc.gpsimd.dma_start(qt_t[0][:], d_qt[0][:])
            nc.gpsimd.dma_start(wv_sb[:], d_wv[:])
            vt_t[0] = vtst.tile([128, 4096], BF16, tag="vt0", name="vts0")
            nc.gpsimd.dma_start(vt_t[0][:], d_vt[0][:])
            # tier 2: first item on each queue gated on a tier-1 tile;
            # the rest follows ungated (concurrent within the tier)
            kt_t[1] = ktst.tile([128, 4096], BF16, tag="kt1", name="kts1")
            gate(kt_t[1], vt_t[0])
            nc.sync.dma_start(kt_t[1][:], d_kt[1][:])
            for n in range(2, 4):
                kt_t[n] = ktst.tile([128, 4096], BF16, tag=f"kt{n}", name=f"kts{n}")
                nc.sync.dma_start(kt_t[n][:], d_kt[n][:])
            nc.sync.dma_start(wo_sb[:], d_wo[:])
            tv = vtst.tile([128, 4096], BF16, tag="vt1", name="vts1")
            gate(tv, qt_t[0])
            nc.gpsimd.dma_start(tv[:], d_vt[1][:])
            vt_t[1] = tv
            for n in range(2, 4):
                tv = vtst.tile([128, 4096], BF16, tag=f"vt{n % 2}", name=f"vts{n}")
                nc.gpsimd.dma_start(tv[:], d_vt[n][:])
                vt_t[n] = tv
            qt_t[1] = qtst.tile([128, 4096], BF16, tag="qt1", name="qts1")
            nc.gpsimd.dma_start(qt_t[1][:], d_qt[1][:])

            def load_late_qt():
                # late q chunks reuse the vt staging buffers; emitted only
                # after every v piece (the buffers' readers) exists
                for n in range(2, 4):
                    t_ = vtst.tile([128, 4096], BF16, tag=f"vt{n % 2}", name=f"qts{n}")
                    nc.sync.dma_start(t_[:], d_qt[n][:])
                    qt_t[n] = t_

            # preload the exp+ln table set while DMA streams
            nc.scalar.activation(warm_sb[:], ones_sb[0:1, 0:2], EXP)
            nc.scalar.activation(warm_sb[:], ones_sb[0:1, 0:2],
                                 mybir.ActivationFunctionType.Ln)

            # denominators of both heads live at partitions 0 and 64 of one
            # [65, 512] tile so a single reciprocal covers both (cost is
            # free-size bound); rows 1..63 are don't-care
            nc.vector.memset(dn2_sb[:], 1.0)

            # bv broadcast over partitions via K=1 matmul
            psbv = ps_acc.tile([128, 512], F32, tag="pacc", name="psbv")
            nc.tensor.matmul(psbv[:], ones_sb[0:1, :], bv_sb[0:1, :])
            nc.vector.tensor_copy(bvbc_sb[:], psbv[:])

            # ---- projection pieces: emitted as two 4-matmul halves --------
            def kq_halves(src_t, w_sb, bias_sb, dst_sb, n, f):
                st = {}

                def h1():
                    accp = ps_acc.tile([128, 512], F32, tag="pacc", name=f"acc{n}{f}")
                    st["acc"] = accp
                    for m in range(4):
                        nc.tensor.matmul(
                            accp[:],
                            w_sb[:, 512 * m + 128 * f:512 * m + 128 * (f + 1)],
                            src_t[n][:, 512 * m:512 * (m + 1)],
                            start=(m == 0), stop=False,
                        )

                def h2():
                    accp = st["acc"]
                    for m in range(4, 8):
                        nc.tensor.matmul(
                            accp[:],
                            w_sb[:, 512 * m + 128 * f:512 * m + 128 * (f + 1)],
                            src_t[n][:, 512 * m:512 * (m + 1)],
                            start=False, stop=(m == 7),
                        )
                    with nc.allow_low_precision(reason="bf16 qT/kT store"):
                        nc.vector.tensor_scalar_add(
                            dst_sb[f][:, 512 * n:512 * (n + 1)],
                            accp[:],
                            bias_sb[:, f:f + 1],
                        )
                return h1, h2

            def kq_piece(src_t, w_sb, bias_sb, dst_sb, n, f):
                h1, h2 = kq_halves(src_t, w_sb, bias_sb, dst_sb, n, f)
                h1()
                h2()

            def v_piece(t):
                q4, o = t // 4, 128 * (t % 4)
                acc = ps_acc.tile([128, 512], F32, tag="pacc", name=f"accv{t}")
                for m in range(8):
                    nc.tensor.matmul(
                        acc[:], vt_t[q4][:, 512 * m + o:512 * m + o + 128],
                        wv_sb[:, 512 * m:512 * (m + 1)],
                        start=(m == 0), stop=(m == 7),
                    )
                v3 = v_sb[t][:].rearrange("p (h e) -> p h e", e=HD + 1)
                nc.vector.memset(v3[:, :, HD:HD + 1], 1.0)
                nc.vector.tensor_add(
                    v3[:, :, 0:HD],
                    acc[:].rearrange("p (h e) -> p h e", e=HD),
                    bvbc_sb[:].rearrange("p (h e) -> p h e", e=HD),
                )

            # ---- wo output projection pieces (chased) ------------------
            wo_pending = []
            oT_cur = [None] * 4
            wo_ost = {}
            wo_nq = [0]

            def emit_wo(count):
                # one call = one [128, 512] half; the merged [128, 1024] store
                # fires after the second half, alternating sync/gpsimd queues
                for _ in range(count):
                    if not wo_pending:
                        return
                    oTs, tt, j = wo_pending.pop(0)
                    pw = ps_acc.tile([128, 512], F32, tag="pacc", name="pw")
                    for f in range(4):
                        nc.tensor.matmul(
                            pw[:], oTs[f][:, 128 * (tt % 4):128 * (tt % 4 + 1)],
                            wo_sb[:, 1024 * f + 512 * j:1024 * f + 512 * (j + 1)],
                            start=(f == 0), stop=(f == 3),
                        )
                    if tt not in wo_ost:
                        wo_ost[tt] = outst.tile([128, 1024], F32, tag="outst", name=f"ost{tt}")
                    ost = wo_ost[tt]
                    nc.vector.tensor_copy(ost[:, 512 * j:512 * (j + 1)], pw[:])
                    if j == 1:
                        eng = nc.sync if wo_nq[0] % 2 == 0 else nc.gpsimd
                        wo_nq[0] += 1
                        eng.dma_start(
                            d_part[128 * tt:128 * (tt + 1), :], wo_ost.pop(tt)[:]
                        )

            # ---- per-group normalization, pipelined ---------------------
            def norm_p1(po, nm):
                oc = ocp.tile([65, 512], F32, tag="oc", name=nm)
                nc.vector.tensor_copy(oc[:], po[0:65, :])
                return oc

            def norm_recip(ocA, ocB):
                # 1/d = exp(-ln d) on the ACT engine (same table set as the
                # softmax exp), freeing the DVE of 3.3us iterative divides
                nc.vector.tensor_copy(dn2_sb[0:1, :], ocA[64:65, :])
                nc.vector.tensor_copy(dn2_sb[64:65, :], ocB[64:65, :])
                ln2 = rcp.tile([65, 512], F32, tag="ln", name="ln2")
                rc2 = rcp.tile([65, 512], F32R, tag="rc", name="rc2")
                with nc.allow_low_precision(reason="recip via exp(-ln d)"):
                    nc.scalar.activation(ln2[:], dn2_sb[:],
                                         mybir.ActivationFunctionType.Ln)
                    nc.scalar.activation(rc2[:], ln2[:], EXP, scale=-1.0)
                return rc2

            def norm_bcast(rc2, i):
                # broadcast head i's reciprocal row to a base-0 [64, 512]
                pb = ps_acc.tile([128, 512], F32, tag="pacc", name=f"pb{i}")
                r = 64 * i
                nc.tensor.matmul(pb[0:64, :], ones_sb[r:r + 1, 0:64], rc2[r:r + 1, :])
                return pb

            def norm_mul(oc, pb, oTf, i):
                with nc.allow_low_precision(reason="bf16 normalized out"):
                    nc.vector.tensor_mul(
                        oTf[64 * i:64 * i + 64, :], oc[0:64, :], pb[0:64, :],
                    )

            # ---- minimal serial head: what group (0,0) m=0..3 needs
            for n4 in range(NQT):
                kq_piece(kt_t, wk_sb, bk_sb, kT_sb, n4, 0)
            kq_piece(qt_t, wq_sb, bq_sb, qT_sb, 0, 0)
            for t in range(4):
                v_piece(t)

            # ---- flat attention pipeline over all (group, m) units -----
            groups = [(n, p) for n in range(NQT) for p in range(NP)]

            def sc_emit(gi, m):
                n, p = groups[gi]
                qsl = slice(512 * n, 512 * (n + 1))
                scp = ps_sc.tile([128, 1024], F32, tag="sc")
                ksl = slice(128 * m, 128 * (m + 1))
                nc.tensor.matmul(
                    scp[:, 0:512], kT_sb[p][0:64, ksl], qT_sb[p][0:64, qsl],
                    tile_position=(0, 0),
                )
                nc.tensor.matmul(
                    scp[:, 512:1024], kT_sb[p][64:128, ksl],
                    qT_sb[p][64:128, qsl], tile_position=(64, 0),
                )
                ex = exch.tile([128, 1024], BF16, tag="ex")
                nc.scalar.activation(ex[:], scp[:], EXP, scale=SCALE)
                return ex

            # chase-slot tables.  kq pieces appear as (h1 slot, h2 slot);
            # deadlines account for the LOOK=3 score lookahead.
            def chase_slots(n, p):
                sl = {}

                def add(m, fn):
                    sl.setdefault(m, []).append(fn)

                if n == 0:
                    if p == 0:
                        h1, h2 = kq_halves(kt_t, wk_sb, bk_sb, kT_sb, 0, 1)
                        add(1, h1); add(2, h2)
                        for t in range(4, 16):
                            add(t - 1, lambda t=t: v_piece(t))
                        h1, h2 = kq_halves(kt_t, wk_sb, bk_sb, kT_sb, 1, 1)
                        add(5, h1); add(6, h2)
                        h1, h2 = kq_halves(qt_t, wq_sb, bq_sb, qT_sb, 0, 1)
                        add(10, h1); add(11, h2)
                    else:
                        h1, h2 = kq_halves(kt_t, wk_sb, bk_sb, kT_sb, 2, p)
                        add(0, h1); add(1, h2)
                        h1, h2 = kq_halves(kt_t, wk_sb, bk_sb, kT_sb, 3, p)
                        add(2, h1); add(3, h2)
                        if p < 3:
                            h1, h2 = kq_halves(kt_t, wk_sb, bk_sb, kT_sb, 0, p + 1)
                            add(4, h1); add(5, h2)
                            h1, h2 = kq_halves(kt_t, wk_sb, bk_sb, kT_sb, 1, p + 1)
                            add(6, h1); add(7, h2)
                            h1, h2 = kq_halves(qt_t, wq_sb, bq_sb, qT_sb, 0, p + 1)
                            add(10, h1); add(11, h2)
                if n + 1 < NQT:
                    h1, h2 = kq_halves(qt_t, wq_sb, bq_sb, qT_sb, n + 1, p)
                    add(10, h1) if (n == 0 and p == 0) else add(8, h1)
                    add(12, h2) if (n == 0 and p == 0) else add(9, h2)
                return sl

            NU = len(groups) * KT
            exq = {}
            for u0 in range(LOOK):
                exq[u0] = sc_emit(u0 // KT, u0 % KT)
            prev_state = None
            norm_st = {}

            for gi, (n, p) in enumerate(groups):
                if gi == 1:
                    load_late_qt()
                if p == 0:
                    oT_cur = [oTp.tile([128, 512], BF16, tag=f"oT{f}", name=f"oT{f}c{n}")
                              for f in range(4)]
                oTs = oT_cur
                poA = ps_pv.tile([128, 512], F32, tag="po", name=f"poA{gi}")
                poB = ps_pv.tile([128, 512], F32, tag="po", name=f"poB{gi}")
                slots = chase_slots(n, p)
                for m in range(KT):
                    u = gi * KT + m
                    if u + LOOK < NU:
                        exq[u + LOOK] = sc_emit((u + LOOK) // KT, (u + LOOK) % KT)
                    ex = exq.pop(u)
                    nc.tensor.matmul(
                        poA[0:65, :], v_sb[m][:, 130 * p:130 * p + 65],
                        ex[:, 0:512], start=(m == 0), stop=(m == KT - 1),
                    )
                    nc.tensor.matmul(
                        poB[0:65, :], v_sb[m][:, 130 * p + 65:130 * p + 130],
                        ex[:, 512:1024], start=(m == 0), stop=(m == KT - 1),
                    )
                    # previous group's norm pipeline in this group's slack
                    if prev_state is not None:
                        pT, pp = prev_state
                        if m == 2:
                            norm_st["rc2"] = norm_recip(norm_st["ocA"], norm_st["ocB"])
                        elif m == 5:
                            norm_st["pbA"] = norm_bcast(norm_st["rc2"], 0)
                        elif m == 6:
                            norm_mul(norm_st["ocA"], norm_st["pbA"], pT[pp], 0)
                        elif m == 7:
                            norm_st["pbB"] = norm_bcast(norm_st["rc2"], 1)
                        elif m == 8:
                            norm_mul(norm_st["ocB"], norm_st["pbB"], pT[pp], 1)
                    for fn in slots.get(m, ()):
                        fn()
                    if m in (9, 10, 12, 13, 14):
                        emit_wo(1)
                # evacuate this group's PV psum right after the stop matmuls
                norm_st["ocA"] = norm_p1(poA, f"ocA{gi}")
                norm_st["ocB"] = norm_p1(poB, f"ocB{gi}")
                prev_state = (oTs, p)
                if p == NP - 1:
                    for t in range(4):
                        for j in range(2):
                            wo_pending.append((oTs, 4 * n + t, j))

            # ---- tail: last group's norms + remaining wo
            pT, pp = prev_state
            rc2 = norm_recip(norm_st["ocA"], norm_st["ocB"])
            pbA = norm_bcast(rc2, 0)
            norm_mul(norm_st["ocA"], pbA, pT[pp], 0)
            pbB = norm_bcast(rc2, 1)
            norm_mul(norm_st["ocB"], pbB, pT[pp], 1)
            emit_wo(len(wo_pending))

    _split_multiwaits(nc)
    return nc


_PROGRAM = None


def _get_program():
    global _PROGRAM
    if _PROGRAM is None:
        _PROGRAM = build_program()
    return _PROGRAM


def _sup(x2d):
    # [nb*128, C] -> [128, nb*C]: row p holds the 128-row blocks' p-th rows
    # concatenated, so a [128, C]-block view is x[:, C*m:C*(m+1)] and one
    # DMA moves the whole tensor with C-sized contiguous lines.
    nb, C = x2d.shape[0] // 128, x2d.shape[1]
    return np.ascontiguousarray(
        x2d.reshape(nb, 128, C).swapaxes(0, 1).reshape(128, nb * C))


def make_in_maps(Q, K, V, Wq, bq, Wk, bk, Wv, bv, Wo, bo):
    import ml_dtypes
    bf = lambda x: np.asarray(x, dtype=np.float32).astype(ml_dtypes.bfloat16)
    f32 = lambda x: np.asarray(x, dtype=np.float32)
    Q, K, V = bf(Q), bf(K), bf(V)
    Wq, Wk, Wv, Wo = bf(Wq), bf(Wk), bf(Wv), bf(Wo)
    bq, bk, bv = f32(bq), f32(bk), f32(bv)
    ones = np.ones((65, 128), np.float32)
    in_maps = []
    for c in range(N_CORES):
        b, hh = c // 2, c % 2
        fs = slice(F * hh, F * (hh + 1))
        qt, kt, vt = Q[b].T, K[b].T, V[b].T
        im = {
            "wq": _sup(Wq[:, fs]),
            "wk": _sup(Wk[:, fs]),
            "wv": _sup(Wv[:, fs]),
            "wo": _sup(Wo[fs, :]),
            "bq": np.ascontiguousarray(bq[fs]),
            "bk": np.ascontiguousarray(bk[fs]),
            "bv": np.ascontiguousarray(bv[fs]),
            "ones": ones,
        }
        for n in range(4):
            sl = slice(512 * n, 512 * (n + 1))
            im[f"qt{n}"] = _sup(qt[:, sl])
            im[f"kt{n}"] = _sup(kt[:, sl])
            im[f"vt{n}"] = _sup(vt[:, sl])
        in_maps.append(im)
    return in_maps


def kernel(Q, K, V, Wq, bq, Wk, bk, Wv, bv, Wo, bo, _trace=False, _trace_kwargs=None):
    nc = _get_program()
    in_maps = make_in_maps(Q, K, V, Wq, bq, Wk, bk, Wv, bv, Wo, bo)
    res = run_bass_kernel_spmd(
        nc, in_maps, core_ids=list(range(N_CORES)),
        trace=_trace, **(_trace_kwargs or {}),
    )
    parts = [r["part"] for r in res.results]
    out = np.stack([parts[2 * b] + parts[2 * b + 1] for b in range(B)])
    out += np.asarray(bo, dtype=np.float32)[None, None, :]
    if _trace:
        return out, res
    return out


# revision 23
# speedup vs baseline: 1.0096x; 1.0086x over previous
"""Multi-head attention (B=4, S=2048, D=1024, H=16) on 8 trn2 NeuronCores.

Sharding: (batch, head-half) -> one core each. Core c handles batch c//2 and
heads (c%2)*8 .. (c%2)*8+7 (feature columns (c%2)*512 .. +512 of the QKV
projections, rows of Wo). Each core computes its 8 heads' attention and a
partial output projection; the host sums the two partials per batch and adds
the output bias.

v4 schedule (vs the 540us baseline):
  - steady state is paced by the ACT engine (256 exps x ~1.05us); scores are
    issued 3 units ahead across group boundaries so ACT rides through PE
    bursts, and all chased projection work is split into 4-matmul halves
    spread over adjacent slots,
  - input DMA: quarter-granularity [128, 512] transfers in exact consumption
    order, split over the two usable DGE queues (sync HW + gpsimd SW; the
    scalar engine carries none, its FIFO would block ACTIVATEs),
  - softmax denominators of both heads are packed at partitions 0/64 of one
    [65, 512] tile -> a single DVE reciprocal per group,
  - wo partials merge into [128, 1024] stores alternating between queues.
Matmuls run in bf16 (fp32 PSUM accumulation); softmax denominators and
reciprocals stay fp32.
"""
import numpy as np

import concourse.bass as bass
import concourse.tile as tile
from concourse import mybir
from concourse.bass_utils import run_bass_kernel_spmd

F32 = mybir.dt.float32
F32R = mybir.dt.float32r
BF16 = mybir.dt.bfloat16
EXP = mybir.ActivationFunctionType.Exp

B, S, DM, H_TOT = 4, 2048, 1024, 16
F = 512          # features per core (8 heads x 64)
HD = 64          # head dim
NH = 8           # heads per core
NP = 4           # head pairs per core
KT = 16          # k tiles of 128
NQT = 4          # q chunks of 512
SCALE = 0.125    # 1/sqrt(64)
N_CORES = 8
LOOK = 4         # scores issued LOOK units ahead

_WAIT_CAP = {"InstEventSemaphore": 2}


def _split_multiwaits(nc):
    """This walrus build accepts 1 sync-wait per instruction (2 on
    EventSemaphore); spread extras over same-engine NOPs placed before."""
    n_spill = 0
    for f in nc.m.functions:
        for bb in f.blocks:
            new = []
            changed = False
            for inst in bb.instructions:
                si = inst.sync_info
                cap = _WAIT_CAP.get(type(inst).__name__, 1)
                if si is not None and len(si.on_wait) > cap:
                    extra = list(si.on_wait[: len(si.on_wait) - cap])
                    del si.on_wait[: len(si.on_wait) - cap]
                    for w in extra:
                        n_spill += 1
                        nop = mybir.InstNoOp(name=f"I-wspill-{n_spill}-{inst.name}")
                        nop.engine = inst.engine
                        nop.sync_info = mybir.SyncInfo(on_wait=[w], on_update=[])
                        new.append(nop)
                    changed = True
                new.append(inst)
            if changed:
                bb.instructions[:] = new
    return n_spill


def build_program():
    nc = bass.Bass("TRN2", target_bir_lowering=False, debug=False, num_devices=1)

    # super-layout inputs: row p of each [128, 4096] tensor holds the
    # per-128-block rows concatenated, so one DMA moves 1 MB with 8 KB
    # contiguous lines (341 GB/s vs ~90 GB/s for 0.125 MB transfers)
    d_qt = [nc.dram_tensor(f"qt{n}", [128, 4096], BF16, kind="ExternalInput").ap() for n in range(4)]
    d_kt = [nc.dram_tensor(f"kt{n}", [128, 4096], BF16, kind="ExternalInput").ap() for n in range(4)]
    d_vt = [nc.dram_tensor(f"vt{n}", [128, 4096], BF16, kind="ExternalInput").ap() for n in range(4)]
    d_wq = nc.dram_tensor("wq", [128, 4096], BF16, kind="ExternalInput").ap()
    d_wk = nc.dram_tensor("wk", [128, 4096], BF16, kind="ExternalInput").ap()
    d_wv = nc.dram_tensor("wv", [128, 4096], BF16, kind="ExternalInput").ap()
    d_wo = nc.dram_tensor("wo", [128, 4096], BF16, kind="ExternalInput").ap()
    d_bq = nc.dram_tensor("bq", [F], F32, kind="ExternalInput").ap()
    d_bk = nc.dram_tensor("bk", [F], F32, kind="ExternalInput").ap()
    d_bv = nc.dram_tensor("bv", [F], F32R, kind="ExternalInput").ap()
    d_ones = nc.dram_tensor("ones", [65, 128], F32R, kind="ExternalInput").ap()
    d_part = nc.dram_tensor("part", [S, DM], F32, kind="ExternalOutput").ap()

    with tile.TileContext(nc) as tc:
        with (
            tc.tile_pool(name="wpool", bufs=1) as wpool,
            tc.tile_pool(name="big", bufs=1) as big,
            tc.tile_pool(name="oTp", bufs=2) as oTp,
            tc.tile_pool(name="ktst", bufs=1) as ktst,
            tc.tile_pool(name="qtst", bufs=1) as qtst,
            tc.tile_pool(name="vtst", bufs=2) as vtst,
            tc.tile_pool(name="exch", bufs=5) as exch,
            tc.tile_pool(name="outst", bufs=2) as outst,
            tc.tile_pool(name="rcp", bufs=2) as rcp,
            tc.tile_pool(name="ocp", bufs=3) as ocp,
            tc.tile_pool(name="ps_sc", bufs=2, space="PSUM") as ps_sc,
            tc.tile_pool(name="ps_pv", bufs=2, space="PSUM") as ps_pv,
            tc.tile_pool(name="ps_acc", bufs=2, space="PSUM") as ps_acc,
        ):
            # ---- resident weight tiles (one merged DMA each)
            wq_sb = wpool.tile([128, 8 * F], BF16, tag="wq")
            wk_sb = wpool.tile([128, 8 * F], BF16, tag="wk")
            wv_sb = wpool.tile([128, 8 * F], BF16, tag="wv")
            wo_sb = wpool.tile([128, 4 * DM], BF16, tag="wo")
            bq_sb = wpool.tile([128, 4], F32, tag="bq")
            bk_sb = wpool.tile([128, 4], F32, tag="bk")
            bv_sb = wpool.tile([1, F], F32R, tag="bv")
            ones_sb = wpool.tile([65, 128], F32R, tag="ones")
            bvbc_sb = wpool.tile([128, F], F32, tag="bvbc")
            warm_sb = wpool.tile([1, 2], F32, tag="warm")
            dn2_sb = wpool.tile([65, 512], F32, tag="dn2")

            # big attention tiles: qT/kT hold the full token range per pair;
            # oT rotates per q-chunk (wo consumption finishes within 1 chunk)
            qT_sb = [big.tile([128, S], BF16, tag=f"qT{f}", name=f"qT{f}") for f in range(4)]
            kT_sb = [big.tile([128, S], BF16, tag=f"kT{f}", name=f"kT{f}") for f in range(4)]
            v_sb = [big.tile([128, NH * (HD + 1)], BF16, tag=f"v{t}", name=f"v{t}") for t in range(KT)]

            # ---- DMA: one 1 MB instruction per staged chunk, in two
            # concurrency tiers (many in-flight transfers are needed to
            # reach aggregate HBM rate; a single transfer only sustains
            # ~90 GB/s).  Tier 1 = everything group (0,0)'s start needs,
            # split across both queues; one gate per queue holds tier 2.
            nc.sync.dma_start(ones_sb[:], d_ones[:])
            nc.sync.dma_start(bq_sb[:], d_bq.rearrange("(f p) -> p f", p=128))
            nc.sync.dma_start(bk_sb[:], d_bk.rearrange("(f p) -> p f", p=128))
            nc.sync.dma_start(bv_sb[:], d_bv.rearrange("(a f) -> a f", a=1))
            kt_t, qt_t, vt_t = {}, {}, {}

            def gate(dst_t, src_t):
                nc.gpsimd.tensor_copy(dst_t[0:1, 0:1], src_t[0:1, 0:1])

            # tier 1 (concurrent): wk,kt0 on sync | wq,qt0,wv,vt0 on gpsimd
            nc.sync.dma_start(wk_sb[:], d_wk[:])
            kt_t[0] = ktst.tile([128, 4096], BF16, tag="kt0", name="kts0")
            nc.sync.dma_start(kt_t[0][:], d_kt[0][:])
            nc.gpsimd.dma_start(wq_sb[:], d_wq[:])
            qt_t[0] = qtst.tile([128, 4096], BF16, tag="qt0", name="qts0")
            nc.gpsimd.dma_start(qt_t[0][:], d_qt[0][:])
            nc.gpsimd.dma_start(wv_sb[:], d_wv[:])
            vt_t[0] = vtst.tile([128, 4096], BF16, tag="vt0", name="vts0")
            nc.gpsimd.dma_start(vt_t[0][:], d_vt[0][:])
            # tier 2: first item on each queue gated on a tier-1 tile;
            # the rest follows ungated (concurrent within the tier)
            kt_t[1] = ktst.tile([128, 4096], BF16, tag="kt1", name="kts1")
            gate(kt_t[1], vt_t[0])
            nc.sync.dma_start(kt_t[1][:], d_kt[1][:])
            for n in range(2, 4):
                kt_t[n] = ktst.tile([128, 4096], BF16, tag=f"kt{n}", name=f"kts{n}")
                nc.sync.dma_start(kt_t[n][:], d_kt[n][:])
            nc.sync.dma_start(wo_sb[:], d_wo[:])
            tv = vtst.tile([128, 4096], BF16, tag="vt1", name="vts1")
            gate(tv, qt_t[0])
            nc.gpsimd.dma_start(tv[:], d_vt[1][:])
            vt_t[1] = tv
            for n in range(2, 4):
                tv = vtst.tile([128, 4096], BF16, tag=f"vt{n % 2}", name=f"vts{n}")
                nc.gpsimd.dma_start(tv[:], d_vt[n][:])
                vt_t[n] = tv
            qt_t[1] = qtst.tile([128, 4096], BF16, tag="qt1", name="qts1")
            nc.gpsimd.dma_start(qt_t[1][:], d_qt[1][:])

            def load_late_qt():
                # late q chunks reuse the vt staging buffers; emitted only
                # after every v piece (the buffers' readers) exists
                for n in range(2, 4):
                    t_ = vtst.tile([128, 4096], BF16, tag=f"vt{n % 2}", name=f"qts{n}")
                    nc.sync.dma_start(t_[:], d_qt[n][:])
                    qt_t[n] = t_

            # preload the exp+ln table set while DMA streams
            nc.scalar.activation(warm_sb[:], ones_sb[0:1, 0:2], EXP)
            nc.scalar.activation(warm_sb[:], ones_sb[0:1, 0:2],
                                 mybir.ActivationFunctionType.Ln)

            # denominators of both heads live at partitions 0 and 64 of one
            # [65, 512] tile so a single reciprocal covers both (cost is
            # free-size bound); rows 1..63 are don't-care
            nc.vector.memset(dn2_sb[:], 1.0)

            # bv broadcast over partitions via K=1 matmul
            psbv = ps_acc.tile([128, 512], F32, tag="pacc", name="psbv")
            nc.tensor.matmul(psbv[:], ones_sb[0:1, :], bv_sb[0:1, :])
            nc.vector.tensor_copy(bvbc_sb[:], psbv[:])

            # ---- projection pieces: emitted as two 4-matmul halves --------
            def kq_halves(src_t, w_sb, bias_sb, dst_sb, n, f):
                st = {}

                def h1():
                    accp = ps_acc.tile([128, 512], F32, tag="pacc", name=f"acc{n}{f}")
                    st["acc"] = accp
                    for m in range(4):
                        nc.tensor.matmul(
                            accp[:],
                            w_sb[:, 512 * m + 128 * f:512 * m + 128 * (f + 1)],
                            src_t[n][:, 512 * m:512 * (m + 1)],
                            start=(m == 0), stop=False,
                        )

                def h2():
                    accp = st["acc"]
                    for m in range(4, 8):
                        nc.tensor.matmul(
                            accp[:],
                            w_sb[:, 512 * m + 128 * f:512 * m + 128 * (f + 1)],
                            src_t[n][:, 512 * m:512 * (m + 1)],
                            start=False, stop=(m == 7),
                        )
                    with nc.allow_low_precision(reason="bf16 qT/kT store"):
                        nc.vector.tensor_scalar_add(
                            dst_sb[f][:, 512 * n:512 * (n + 1)],
                            accp[:],
                            bias_sb[:, f:f + 1],
                        )
                return h1, h2

            def kq_piece(src_t, w_sb, bias_sb, dst_sb, n, f):
                h1, h2 = kq_halves(src_t, w_sb, bias_sb, dst_sb, n, f)
                h1()
                h2()

            def v_piece(t):
                q4, o = t // 4, 128 * (t % 4)
                acc = ps_acc.tile([128, 512], F32, tag="pacc", name=f"accv{t}")
                for m in range(8):
                    nc.tensor.matmul(
                        acc[:], vt_t[q4][:, 512 * m + o:512 * m + o + 128],
                        wv_sb[:, 512 * m:512 * (m + 1)],
                        start=(m == 0), stop=(m == 7),
                    )
                v3 = v_sb[t][:].rearrange("p (h e) -> p h e", e=HD + 1)
                nc.vector.memset(v3[:, :, HD:HD + 1], 1.0)
                nc.vector.tensor_add(
                    v3[:, :, 0:HD],
                    acc[:].rearrange("p (h e) -> p h e", e=HD),
                    bvbc_sb[:].rearrange("p (h e) -> p h e", e=HD),
                )

            # ---- wo output projection pieces (chased) ------------------
            wo_pending = []
            oT_cur = [None] * 4
            wo_ost = {}
            wo_nq = [0]

            def emit_wo(count):
                # one call = one [128, 512] half; the merged [128, 1024] store
                # fires after the second half, alternating sync/gpsimd queues
                for _ in range(count):
                    if not wo_pending:
                        return
                    oTs, tt, j = wo_pending.pop(0)
                    pw = ps_acc.tile([128, 512], F32, tag="pacc", name="pw")
                    for f in range(4):
                        nc.tensor.matmul(
                            pw[:], oTs[f][:, 128 * (tt % 4):128 * (tt % 4 + 1)],
                            wo_sb[:, 1024 * f + 512 * j:1024 * f + 512 * (j + 1)],
                            start=(f == 0), stop=(f == 3),
                        )
                    if tt not in wo_ost:
                        wo_ost[tt] = outst.tile([128, 1024], F32, tag="outst", name=f"ost{tt}")
                    ost = wo_ost[tt]
                    nc.vector.tensor_copy(ost[:, 512 * j:512 * (j + 1)], pw[:])
                    if j == 1:
                        eng = nc.sync if wo_nq[0] % 2 == 0 else nc.gpsimd
                        wo_nq[0] += 1
                        eng.dma_start(
                            d_part[128 * tt:128 * (tt + 1), :], wo_ost.pop(tt)[:]
                        )

            # ---- per-group normalization, pipelined ---------------------
            def norm_p1(po, nm):
                oc = ocp.tile([65, 512], F32, tag="oc", name=nm)
                nc.vector.tensor_copy(oc[:], po[0:65, :])
                return oc

            def norm_recip(ocA, ocB):
                # 1/d = exp(-ln d) on the ACT engine (same table set as the
                # softmax exp), freeing the DVE of 3.3us iterative divides
                nc.vector.tensor_copy(dn2_sb[0:1, :], ocA[64:65, :])
                nc.vector.tensor_copy(dn2_sb[64:65, :], ocB[64:65, :])
                ln2 = rcp.tile([65, 512], F32, tag="ln", name="ln2")
                rc2 = rcp.tile([65, 512], F32R, tag="rc", name="rc2")
                with nc.allow_low_precision(reason="recip via exp(-ln d)"):
                    nc.scalar.activation(ln2[:], dn2_sb[:],
                                         mybir.ActivationFunctionType.Ln)
                    nc.scalar.activation(rc2[:], ln2[:], EXP, scale=-1.0)
                return rc2

            def norm_bcast(rc2, i):
                # broadcast head i's reciprocal row to a base-0 [64, 512]
                pb = ps_acc.tile([128, 512], F32, tag="pacc", name=f"pb{i}")
                r = 64 * i
                nc.tensor.matmul(pb[0:64, :], ones_sb[r:r + 1, 0:64], rc2[r:r + 1, :])
                return pb

            def norm_mul(oc, pb, oTf, i):
                with nc.allow_low_precision(reason="bf16 normalized out"):
                    nc.vector.tensor_mul(
                        oTf[64 * i:64 * i + 64, :], oc[0:64, :], pb[0:64, :],
                    )

            # ---- minimal serial head: what group (0,0) m=0..3 needs
            for n4 in range(NQT):
                kq_piece(kt_t, wk_sb, bk_sb, kT_sb, n4, 0)
            kq_piece(qt_t, wq_sb, bq_sb, qT_sb, 0, 0)
            for t in range(4):
                v_piece(t)

            # ---- flat attention pipeline over all (group, m) units -----
            groups = [(n, p) for n in range(NQT) for p in range(NP)]

            def sc_emit(gi, m):
                n, p = groups[gi]
                qsl = slice(512 * n, 512 * (n + 1))
                scp = ps_sc.tile([128, 1024], F32, tag="sc")
                ksl = slice(128 * m, 128 * (m + 1))
                nc.tensor.matmul(
                    scp[:, 0:512], kT_sb[p][0:64, ksl], qT_sb[p][0:64, qsl],
                    tile_position=(0, 0),
                )
                nc.tensor.matmul(
                    scp[:, 512:1024], kT_sb[p][64:128, ksl],
                    qT_sb[p][64:128, qsl], tile_position=(64, 0),
                )
                ex = exch.tile([128, 1024], BF16, tag="ex")
                nc.scalar.activation(ex[:], scp[:], EXP, scale=SCALE)
                return ex

            # chase-slot tables.  kq pieces appear as (h1 slot, h2 slot);
            # deadlines account for the LOOK=3 score lookahead.
            def chase_slots(n, p):
                sl = {}

                def add(m, fn):
                    sl.setdefault(m, []).append(fn)

                if n == 0:
                    if p == 0:
                        h1, h2 = kq_halves(kt_t, wk_sb, bk_sb, kT_sb, 0, 1)
                        add(1, h1); add(2, h2)
                        for t in range(4, 16):
                            add(t - 1, lambda t=t: v_piece(t))
                        h1, h2 = kq_halves(kt_t, wk_sb, bk_sb, kT_sb, 1, 1)
                        add(5, h1); add(6, h2)
                        h1, h2 = kq_halves(qt_t, wq_sb, bq_sb, qT_sb, 0, 1)
                        add(10, h1); add(11, h2)
                    else:
                        h1, h2 = kq_halves(kt_t, wk_sb, bk_sb, kT_sb, 2, p)
                        add(0, h1); add(1, h2)
                        h1, h2 = kq_halves(kt_t, wk_sb, bk_sb, kT_sb, 3, p)
                        add(2, h1); add(3, h2)
                        if p < 3:
                            h1, h2 = kq_halves(kt_t, wk_sb, bk_sb, kT_sb, 0, p + 1)
                            add(4, h1); add(5, h2)
                            h1, h2 = kq_halves(kt_t, wk_sb, bk_sb, kT_sb, 1, p + 1)
                            add(6, h1); add(7, h2)
                            h1, h2 = kq_halves(qt_t, wq_sb, bq_sb, qT_sb, 0, p + 1)
                            add(10, h1); add(11, h2)
                if n + 1 < NQT:
                    h1, h2 = kq_halves(qt_t, wq_sb, bq_sb, qT_sb, n + 1, p)
                    add(13, h1) if (n == 0 and p == 0) else add(3, h1)
                    add(14, h2) if (n == 0 and p == 0) else add(4, h2)
                return sl

            NU = len(groups) * KT
            exq = {}
            for u0 in range(LOOK):
                exq[u0] = sc_emit(u0 // KT, u0 % KT)
            prev_state = None
            norm_st = {}

            for gi, (n, p) in enumerate(groups):
                if gi == 1:
                    load_late_qt()
                if p == 0:
                    oT_cur = [oTp.tile([128, 512], BF16, tag=f"oT{f}", name=f"oT{f}c{n}")
                              for f in range(4)]
                oTs = oT_cur
                poA = ps_pv.tile([128, 512], F32, tag="po", name=f"poA{gi}")
                poB = ps_pv.tile([128, 512], F32, tag="po", name=f"poB{gi}")
                slots = chase_slots(n, p)
                for m in range(KT):
                    u = gi * KT + m
                    if u + LOOK < NU:
                        exq[u + LOOK] = sc_emit((u + LOOK) // KT, (u + LOOK) % KT)
                    ex = exq.pop(u)
                    nc.tensor.matmul(
                        poA[0:65, :], v_sb[m][:, 130 * p:130 * p + 65],
                        ex[:, 0:512], start=(m == 0), stop=(m == KT - 1),
                    )
                    nc.tensor.matmul(
                        poB[0:65, :], v_sb[m][:, 130 * p + 65:130 * p + 130],
                        ex[:, 512:1024], start=(m == 0), stop=(m == KT - 1),
                    )
                    # previous group's norm pipeline in this group's slack
                    if prev_state is not None:
                        pT, pp = prev_state
                        if m == 2:
                            norm_st["rc2"] = norm_recip(norm_st["ocA"], norm_st["ocB"])
                        elif m == 5:
                            norm_st["pbA"] = norm_bcast(norm_st["rc2"], 0)
                        elif m == 6:
                            norm_mul(norm_st["ocA"], norm_st["pbA"], pT[pp], 0)
                        elif m == 7:
                            norm_st["pbB"] = norm_bcast(norm_st["rc2"], 1)
                        elif m == 8:
                            norm_mul(norm_st["ocB"], norm_st["pbB"], pT[pp], 1)
                    for fn in slots.get(m, ()):
                        fn()
                    if m in (9, 10, 12, 13, 14):
                        emit_wo(1)
                # evacuate this group's PV psum right after the stop matmuls
                norm_st["ocA"] = norm_p1(poA, f"ocA{gi}")
                norm_st["ocB"] = norm_p1(poB, f"ocB{gi}")
                prev_state = (oTs, p)
                if p == NP - 1:
                    for t in range(4):
                        for j in range(2):
                            wo_pending.append((oTs, 4 * n + t, j))

            # ---- tail: last group's norms + remaining wo
            pT, pp = prev_state
            rc2 = norm_recip(norm_st["ocA"], norm_st["ocB"])
            pbA = norm_bcast(rc2, 0)
            norm_mul(norm_st["ocA"], pbA, pT[pp], 0)
            pbB = norm_bcast(rc2, 1)
            norm_mul(norm_st["ocB"], pbB, pT[pp], 1)
            emit_wo(len(wo_pending))

    _split_multiwaits(nc)
    return nc


_PROGRAM = None


def _get_program():
    global _PROGRAM
    if _PROGRAM is None:
        _PROGRAM = build_program()
    return _PROGRAM


def _sup(x2d):
    # [nb*128, C] -> [128, nb*C]: row p holds the 128-row blocks' p-th rows
    # concatenated, so a [128, C]-block view is x[:, C*m:C*(m+1)] and one
    # DMA moves the whole tensor with C-sized contiguous lines.
    nb, C = x2d.shape[0] // 128, x2d.shape[1]
    return np.ascontiguousarray(
        x2d.reshape(nb, 128, C).swapaxes(0, 1).reshape(128, nb * C))


def make_in_maps(Q, K, V, Wq, bq, Wk, bk, Wv, bv, Wo, bo):
    import ml_dtypes
    bf = lambda x: np.asarray(x, dtype=np.float32).astype(ml_dtypes.bfloat16)
    f32 = lambda x: np.asarray(x, dtype=np.float32)
    Q, K, V = bf(Q), bf(K), bf(V)
    Wq, Wk, Wv, Wo = bf(Wq), bf(Wk), bf(Wv), bf(Wo)
    bq, bk, bv = f32(bq), f32(bk), f32(bv)
    ones = np.ones((65, 128), np.float32)
    in_maps = []
    for c in range(N_CORES):
        b, hh = c // 2, c % 2
        fs = slice(F * hh, F * (hh + 1))
        qt, kt, vt = Q[b].T, K[b].T, V[b].T
        im = {
            "wq": _sup(Wq[:, fs]),
            "wk": _sup(Wk[:, fs]),
            "wv": _sup(Wv[:, fs]),
            "wo": _sup(Wo[fs, :]),
            "bq": np.ascontiguousarray(bq[fs]),
            "bk": np.ascontiguousarray(bk[fs]),
            "bv": np.ascontiguousarray(bv[fs]),
            "ones": ones,
        }
        for n in range(4):
            sl = slice(512 * n, 512 * (n + 1))
            im[f"qt{n}"] = _sup(qt[:, sl])
            im[f"kt{n}"] = _sup(kt[:, sl])
            im[f"vt{n}"] = _sup(vt[:, sl])
        in_maps.append(im)
    return in_maps


def kernel(Q, K, V, Wq, bq, Wk, bk, Wv, bv, Wo, bo, _trace=False, _trace_kwargs=None):
    nc = _get_program()
    in_maps = make_in_maps(Q, K, V, Wq, bq, Wk, bk, Wv, bv, Wo, bo)
    res = run_bass_kernel_spmd(
        nc, in_maps, core_ids=list(range(N_CORES)),
        trace=_trace, **(_trace_kwargs or {}),
    )
    parts = [r["part"] for r in res.results]
    out = np.stack([parts[2 * b] + parts[2 * b + 1] for b in range(B)])
    out += np.asarray(bo, dtype=np.float32)[None, None, :]
    if _trace:
        return out, res
    return out
